# revision 10
# baseline (speedup 1.0000x reference)
"""Trainium2 Bass kernel for NeighbourAssignment GNN message passing.

Full-input contract: kernel(**inputs) takes the complete arrays and returns
the complete [N, K, OUT] output. Internally shards data-parallel over the
node dimension across 8 NeuronCores; src and the small weights are
replicated on every core.
"""

import sys
import functools

sys.path.insert(0, "/opt/trn_rl_repo")

import numpy as np

# Problem constants (hardcoded per harness contract).
N = 50000
K = 16
C = 64          # CS == CT == 64
S = 4
OUT = 64
N_CORES = 8
P = 128
NPC = N // N_CORES            # 6250 nodes per core
NT = (NPC + P - 1) // P       # 49 node tiles per core
NPAD = NT * P                 # 6272 padded nodes per core
STRIDE = C + 4                # 68: gather chunk stride; 4-wide ones gap


def _build_program(nt: int, n_src: int):
    import concourse.bass as bass
    import concourse.bacc as bacc
    import concourse.mybir as mybir
    import concourse.tile as tile
    from concourse.masks import make_identity
    from contextlib import ExitStack

    fp32 = mybir.dt.float32
    i32 = mybir.dt.int32
    npad = nt * P

    nc = bacc.Bacc("TRN2", num_devices=N_CORES, debug=False)

    # x carries a baked-in ones column (C+1); src carries a 4-wide ones pad
    # (STRIDE) so transposes/gathers have a single producer each (the ISA has
    # one sync-wait slot per instruction; multi-producer matmul inputs make
    # walrus fail with "Too many sync wait commands").
    x_d = nc.dram_tensor("x", [npad, C + 1], fp32, kind="ExternalInput")
    idx_d = nc.dram_tensor("nidx", [npad, K], i32, kind="ExternalInput")
    src_d = nc.dram_tensor("src", [n_src, STRIDE], fp32, kind="ExternalInput")
    ws_d = nc.dram_tensor("ws_aug", [C + 1, S], fp32, kind="ExternalInput")
    wl_d = nc.dram_tensor("wlin_int", [C + 1, S * OUT], fp32, kind="ExternalInput")
    wt_d = nc.dram_tensor("wt_aug", [C + 1, 2 * S], fp32, kind="ExternalInput")
    out_d = nc.dram_tensor("out", [npad, K * OUT], fp32, kind="ExternalOutput")

    Copy = mybir.ActivationFunctionType.Copy
    Exp = mybir.ActivationFunctionType.Exp
    AX = mybir.AxisListType.X

    def bcast(ap, count, at=None):
        # Append (or insert) a zero-stride dim of length `count` to an AP.
        new = ap.ap.copy()
        if at is None:
            new.append([0, count])
        else:
            new.insert(at, [0, count])
        return bass.AP(ap.tensor, ap.offset, new)

    with tile.TileContext(nc) as tc, ExitStack() as ctx:
        const = ctx.enter_context(tc.tile_pool(name="const", bufs=1))
        sbt = ctx.enter_context(tc.tile_pool(name="sbt", bufs=3))      # per node tile
        sbs = ctx.enter_context(tc.tile_pool(name="sbs", bufs=4))      # per strip
        ps_tr = ctx.enter_context(tc.tile_pool(name="ps_tr", bufs=2, space="PSUM"))
        ps_l = ctx.enter_context(tc.tile_pool(name="ps_l", bufs=2, space="PSUM"))
        ps_w = ctx.enter_context(tc.tile_pool(name="ps_w", bufs=3, space="PSUM"))

        ident = const.tile([P, P], fp32)
        make_identity(nc, ident[:])
        ws_sb = const.tile([C + 1, S], fp32)
        nc.sync.dma_start(ws_sb[:], ws_d.ap()[:, :])
        wl_sb = const.tile([C + 1, S * OUT], fp32)
        nc.sync.dma_start(wl_sb[:], wl_d.ap()[:, :])
        wt_sb = const.tile([C + 1, 2 * S], fp32)
        nc.sync.dma_start(wt_sb[:], wt_d.ap()[:, :])

        # Warmup matmuls: gate each one-time producer (const DMAs, identity)
        # behind a tiny PE op so steady-state matmuls never need more than
        # one fresh sync wait.
        warm = ps_l.tile([P, 2 * S], fp32, tag="psl")
        for wi, g in enumerate([ident, ws_sb, wl_sb, wt_sb]):
            nc.tensor.matmul(warm[0:1, wi:wi + 1], lhsT=g[0:1, 0:1],
                             rhs=g[0:1, 0:1], start=True, stop=True,
                             skip_group_check=True)

        for it in range(nt):
            nb = it * P

            idx_t = sbt.tile([P, K], i32, tag="idx")
            nc.sync.dma_start(idx_t[:], idx_d.ap()[nb:nb + P, :])

            # One indirect DMA per neighbor slot: HW consumes exactly one
            # offset per partition and walks the source AP for the rest, so
            # a [128, K] offset AP does NOT gather K rows per partition.
            msgs = sbt.tile([P, K * STRIDE], fp32, tag="msgs")
            for k in range(K):
                nc.gpsimd.indirect_dma_start(
                    out=msgs[:, k * STRIDE:(k + 1) * STRIDE],
                    out_offset=None,
                    in_=src_d.ap()[:, :],
                    in_offset=bass.IndirectOffsetOnAxis(
                        ap=idx_t[:, k:k + 1], axis=0),
                )

            x_t = sbt.tile([P, C + 1], fp32, tag="x")
            nc.sync.dma_start(x_t[:, :], x_d.ap()[nb:nb + P, :])
            xT_ps = ps_tr.tile([P, P], fp32, tag="tr")
            nc.tensor.transpose(out=xT_ps[0:C + 1, :], in_=x_t[:, :], identity=ident[:])
            xT = sbt.tile([C + 1, P], fp32, tag="xT")
            nc.scalar.activation(xT[:], xT_ps[0:C + 1, :], Copy)

            out_t = sbt.tile([P, K * OUT], fp32, tag="out")

            for j in range(K // 2):
                psl = ps_l.tile([P, 2 * S], fp32, tag="psl")
                psw = ps_w.tile([P, 2 * S * OUT], fp32, tag="psw")
                # t (= x@Wt + biases) first: start=True may only be used by
                # the first writer of a PSUM bank (zero-region semantics).
                nc.tensor.matmul(
                    psl[:, :], lhsT=xT[:], rhs=wt_sb[:],
                    start=True, stop=False, skip_group_check=True,
                )
                for h in range(2):
                    k = 2 * j + h
                    mT_ps = ps_tr.tile([P, P], fp32, tag="tr")
                    nc.tensor.transpose(
                        out=mT_ps[0:C + 1, :],
                        in_=msgs[:, k * STRIDE:k * STRIDE + C + 1],
                        identity=ident[:],
                    )
                    mT = sbs.tile([C + 1, P], fp32, tag="mT")
                    nc.scalar.activation(mT[:], mT_ps[0:C + 1, :], Copy)
                    nc.tensor.matmul(
                        psl[:, h * S:(h + 1) * S], lhsT=mT[:], rhs=ws_sb[:],
                        start=False, stop=(h == 1), skip_group_check=True,
                    )
                    nc.tensor.matmul(
                        psw[:, h * S * OUT:(h + 1) * S * OUT], lhsT=mT[:], rhs=wl_sb[:],
                        start=(h == 0), stop=(h == 1), skip_group_check=True,
                    )

                # softmax (no max subtraction; logits are O(1)) ------------
                e = sbs.tile([P, 2 * S], fp32, tag="e")
                nc.scalar.activation(e[:], psl[:, :], Exp)
                d = sbs.tile([P, 2], fp32, tag="d")
                nc.vector.tensor_reduce(
                    d[:], e[:, :].rearrange("p (t s) -> p t s", t=2),
                    axis=AX, op=mybir.AluOpType.add,
                )
                r = sbs.tile([P, 2], fp32, tag="r")
                nc.vector.reciprocal(r[:], d[:])
                a = sbs.tile([P, 2 * S], fp32, tag="a")
                nc.vector.tensor_tensor(
                    out=a[:, :].rearrange("p (t s) -> p t s", t=2),
                    in0=e[:, :].rearrange("p (t s) -> p t s", t=2),
                    in1=bcast(r[:, :], S),
                    op=mybir.AluOpType.mult,
                )

                # weighted sum over S (psw is (o,s)-interleaved) ----------
                tmp = sbs.tile([P, 2 * S * OUT], fp32, tag="tmp")
                a_v = bcast(a[:, :].rearrange("p (t s) -> p t s", t=2), OUT, at=2)
                nc.vector.tensor_tensor(
                    out=tmp[:, :].rearrange("p (t o s) -> p t o s", t=2, s=S),
                    in0=psw[:, :].rearrange("p (t o s) -> p t o s", t=2, s=S),
                    in1=a_v,
                    op=mybir.AluOpType.mult,
                )
                nc.vector.tensor_reduce(
                    out_t[:, j * 2 * OUT:(j + 1) * 2 * OUT].rearrange(
                        "p (t o) -> p t o", t=2),
                    tmp[:, :].rearrange("p (t o s) -> p t o s", t=2, s=S),
                    axis=AX, op=mybir.AluOpType.add,
                )

            nc.sync.dma_start(out_d.ap()[nb:nb + P, :], out_t[:])

    nc.compile()
    return nc


@functools.lru_cache(maxsize=2)
def _get_program(nt: int, n_src: int):
    return _build_program(nt, n_src)


def _prep_weights(Wt, bt, Ws, bs, W_lin, b_lin):
    ws_aug = np.zeros((C + 1, S), np.float32)
    ws_aug[:C] = Ws
    ws_aug[C] = bs + bt
    # wlin_int col (o*S + s) = W_lin[s, :, o] / S ; bias row = b_lin[s, o] / S
    wlin_int = np.zeros((C + 1, S * OUT), np.float32)
    wlin_int[:C] = (np.transpose(W_lin, (1, 2, 0)) / S).reshape(C, OUT * S)
    wlin_int[C] = (b_lin.T / S).reshape(OUT * S)
    wt_aug = np.zeros((C + 1, 2 * S), np.float32)
    wt_aug[:C] = np.concatenate([Wt, Wt], axis=1)
    return ws_aug, wlin_int, wt_aug


def kernel(x, src, neighbor_idx, Wt, bt, Ws, bs, W_lin, b_lin, _trace=False):
    from concourse import bass_utils

    x = np.asarray(x, np.float32)
    src = np.asarray(src, np.float32)
    src_pad = np.ones((src.shape[0], STRIDE), np.float32)
    src_pad[:, :C] = src
    neighbor_idx = np.asarray(neighbor_idx, np.int32)
    ws_aug, wlin_int, wt_aug = _prep_weights(
        np.asarray(Wt, np.float32), np.asarray(bt, np.float32),
        np.asarray(Ws, np.float32), np.asarray(bs, np.float32),
        np.asarray(W_lin, np.float32), np.asarray(b_lin, np.float32))

    nc = _get_program(NT, N)

    in_maps = []
    for c in range(N_CORES):
        lo = c * NPC
        xs = np.ones((NPAD, C + 1), np.float32)
        xs[:NPC, :C] = x[lo:lo + NPC]
        xs[NPC:, :C] = 0.0
        ids = np.zeros((NPAD, K), np.int32)
        ids[:NPC] = neighbor_idx[lo:lo + NPC]
        in_maps.append({
            "x": xs, "nidx": ids, "src": src_pad,
            "ws_aug": ws_aug, "wlin_int": wlin_int, "wt_aug": wt_aug,
        })

    res = bass_utils.run_bass_kernel_spmd(
        nc, in_maps, core_ids=list(range(N_CORES)), trace=_trace)

    out = np.empty((N, K, OUT), np.float32)
    for c in range(N_CORES):
        out[c * NPC:(c + 1) * NPC] = (
            res.results[c]["out"][:NPC].reshape(NPC, K, OUT))
    if _trace:
        kernel._last_results = res
    return out
